# revision 36
# baseline (speedup 1.0000x reference)
"""Trainium2 Bass kernel: GQA attention block (QKV proj + RMSNorm + RoPE +
bidirectional attention + output proj), 8-way data-parallel.

Sharding: 8 cores = 4 batches x 2 query-token halves. Each core computes
K/V for its full batch (1024 tokens) and attention + o_proj for its 512
query tokens. No inter-core communication; host gathers the 8 output shards.

Per-core kernel (all matmuls in bf16, fp32 accumulation):
  P1  K/V projection, RMSNorm+RoPE on K, PE-transpose K -> ktT [d, h, t]
  P2  Q projection, RMSNorm+RoPE, PE-transpose -> qT [d, h, t]
      interleaved with attention per 4-head group:
        scores^T [k, q] = ktT_blk.T @ qT   (per 128-key block)
        p = exp(scale * scores)            (ScalarE, bf16)
        Z [1, q]  = ones.T @ p             (PE matmul, M=1)
        rz = 1/Z                           (DVE), bcast to 128 partitions (DMA)
        avT [d, q] = sum_k V_blk.T @ p     (PE)
        aT[:, h, :] = avT * rz             (DVE, evict to bf16)
  P3  o_proj: y [t, o] = aT.T @ woT, fp32 out
"""

import os
import sys
from contextlib import ExitStack

for _p in (
    "/root/.axon_site",
    "/root/.axon_site/_ro/trn_rl_repo",
    "/root/.axon_site/_ro/pypackages",
    "/opt/trn_rl_repo",
):
    if os.path.isdir(_p) and _p not in sys.path:
        sys.path.append(_p)

import ml_dtypes
import numpy as np

import concourse.bacc as bacc
import concourse.bass as bass
import concourse.tile as tile
from concourse import bass_isa, mybir
from concourse.bass_utils import run_bass_kernel_spmd
from concourse.masks import make_identity

BF16 = mybir.dt.bfloat16
F32 = mybir.dt.float32
AF = mybir.ActivationFunctionType
OP = mybir.AluOpType
AX = mybir.AxisListType

B = 4
S = 1024
SQ = 512            # query tokens per core
HIDDEN = 4096
NH = 32
NKV = 8
HD = 128
EPS = 1e-6
ROPE_BASE = 1000000.0
SCALE = float(HD) ** -0.5
NDT = HIDDEN // 128  # 32 contraction tiles
N_CORES = 8

_BF = ml_dtypes.bfloat16


def _bcast_mid(ap, n):
    """[P, X...] -> [P, n, X...] with a stride-0 middle dim."""
    return bass.AP(tensor=ap.tensor, offset=ap.offset, ap=[ap.ap[0], [0, n], *ap.ap[1:]])


def build_bass() -> bass.Bass:
    nc = bacc.Bacc("TRN2", target_bir_lowering=False, debug=False, num_devices=N_CORES)

    # DRAM I/O (per core). hs blocks pre-arranged on host as [tile, p, a, t]
    # so each DMA is one contiguous 1MB read.
    hs_kv = nc.declare_dram_parameter("hs_kv", [8, 128, NDT, 128], BF16, isOutput=False)
    hs_q = nc.declare_dram_parameter("hs_q", [4, 128, NDT, 128], BF16, isOutput=False)
    wkvT = nc.declare_dram_parameter("wkvT", [HIDDEN, 2048], BF16, isOutput=False)
    wqT = nc.declare_dram_parameter("wqT", [HIDDEN, HIDDEN], BF16, isOutput=False)
    woT = nc.declare_dram_parameter("woT", [HIDDEN, HIDDEN], BF16, isOutput=False)
    # rope tables [t, cA|sA|cB|sB] (cos/sin with rms-norm weight folded in)
    ropeq = nc.declare_dram_parameter("ropeq", [SQ, 256], F32, isOutput=False)
    ropek = nc.declare_dram_parameter("ropek", [S, 256], F32, isOutput=False)
    y = nc.declare_dram_parameter("y", [SQ, HIDDEN], F32, isOutput=True)

    with ExitStack() as ctx:
        tc = ctx.enter_context(tile.TileContext(nc))

        persist = ctx.enter_context(tc.tile_pool(name="persist", bufs=1))
        ktT = persist.tile([128, NKV, S], BF16, tag="ktT")        # [d, kvh, t]
        v_all = persist.tile([128, 8, NKV, 128], BF16, tag="v")   # [t%128, tt, kvh, d]
        aT = persist.tile([128, NH, SQ], BF16, tag="aT")          # [d, h, q]
        tabq = persist.tile([128, 4, 256], F32, tag="tabq")
        tabk = persist.tile([128, 8, 256], F32, tag="tabk")
        ident = persist.tile([128, 128], BF16, tag="ident")

        wp = ctx.enter_context(tc.tile_pool(name="wp", bufs=48))
        hp = ctx.enter_context(tc.tile_pool(name="hp", bufs=2))
        scratch = ctx.enter_context(tc.tile_pool(name="scratch", bufs=2))
        qtp = ctx.enter_context(tc.tile_pool(name="qtp", bufs=3))
        qnp = ctx.enter_context(tc.tile_pool(name="qnp", bufs=8))
        attn_sb = ctx.enter_context(tc.tile_pool(name="attn_sb", bufs=2))
        ysb = ctx.enter_context(tc.tile_pool(name="ysb", bufs=2))

        pp_ps = ctx.enter_context(tc.tile_pool(name="pp_ps", bufs=2, space="PSUM"))
        st_ps = ctx.enter_context(tc.tile_pool(name="st_ps", bufs=2, space="PSUM"))
        av_ps = ctx.enter_context(tc.tile_pool(name="av_ps", bufs=2, space="PSUM"))

        def load_w_tiles(wsrc, col0):
            """32 [128, 512] rhs tiles covering rows 0..4096, cols col0:col0+512."""
            tiles = []
            for a in range(NDT):
                wt = wp.tile([128, 512], BF16, tag="wt")
                nc.sync.dma_start(
                    out=wt[:], in_=wsrc[a * 128:(a + 1) * 128, col0:col0 + 512]
                )
                tiles.append(wt)
            return tiles

        # First hs block + first weight chunk first: nothing blocks the PE
        # longer than these at kernel start. The hs block is split into 8
        # sub-DMAs so it spreads across queues instead of one 1MB transfer.
        hs_first = hp.tile([128, NDT, 128], BF16, tag="hs")
        for part in range(8):
            nc.sync.dma_start(out=hs_first[:, part * 4:(part + 1) * 4, :],
                              in_=hs_kv[0][:, part * 4:(part + 1) * 4, :])
        wts_first = load_w_tiles(wkvT, 0)
        make_identity(nc, ident[:])
        nc.sync.dma_start(out=tabq[:], in_=ropeq[:].rearrange("(a p) c -> p a c", p=128))
        nc.sync.dma_start(out=tabk[:], in_=ropek[:].rearrange("(a p) c -> p a c", p=128))

        def norm_rope(ps, tab_tile, tt, qn):
            """RMSNorm + RoPE on a [128 tok, 4 heads, 128] psum projection,
            into bf16 qn [128, 4, 128]."""
            psv = ps[:].rearrange("p (h d) -> p h d", h=4)
            qf = scratch.tile([128, 4, 128], F32, tag="qf")
            qsq = scratch.tile([128, 512], BF16, tag="qsq")
            ssq = scratch.tile([128, 4], F32, tag="ssq")
            rr = scratch.tile([128, 4], F32, tag="rr")
            t1 = scratch.tile([128, 4, 64], F32, tag="t1")
            t2 = scratch.tile([128, 4, 64], F32, tag="t2")
            t3 = scratch.tile([128, 4, 64], F32, tag="t1")
            t4 = scratch.tile([128, 4, 64], F32, tag="t2")

            nc.scalar.copy(out=qf[:], in_=psv)
            nc.scalar.activation(out=qsq[:], in_=ps[:], func=AF.Square)
            nc.vector.reduce_sum(
                out=ssq[:], in_=qsq[:].rearrange("p (h d) -> p h d", h=4), axis=AX.X
            )
            # v = ssq/128 + eps, then r = rsqrt(v) via bit-trick seed + 2 Newton
            # iterations (all-DVE; keeps ScalarE on a single ACT table set).
            vv = scratch.tile([128, 4], F32, tag="vv")
            rt = scratch.tile([128, 4], F32, tag="rt")
            nc.vector.tensor_scalar(out=vv[:], in0=ssq[:], scalar1=1.0 / HD,
                                    scalar2=EPS, op0=OP.mult, op1=OP.add)
            vi = vv[:].bitcast(mybir.dt.int32)
            ri = rr[:].bitcast(mybir.dt.int32)
            nc.vector.tensor_scalar(out=ri, in0=vi, scalar1=1, scalar2=None,
                                    op0=OP.arith_shift_right)
            nc.vector.tensor_scalar(out=ri, in0=ri, scalar1=-1, scalar2=0x5F3759DF,
                                    op0=OP.mult, op1=OP.add)
            for _ in range(2):
                nc.vector.tensor_mul(rt[:], rr[:], rr[:])
                nc.vector.tensor_mul(rt[:], rt[:], vv[:])
                nc.vector.tensor_scalar(out=rt[:], in0=rt[:], scalar1=-0.5,
                                        scalar2=1.5, op0=OP.mult, op1=OP.add)
                nc.vector.tensor_mul(rr[:], rr[:], rt[:])
            for hh in range(4):
                nc.vector.tensor_scalar_mul(qf[:, hh, :], qf[:, hh, :], rr[:, hh:hh + 1])
            q1 = qf[:, :, 0:64]
            q2 = qf[:, :, 64:128]
            cA = _bcast_mid(tab_tile[:, tt, 0:64], 4)
            sA = _bcast_mid(tab_tile[:, tt, 64:128], 4)
            cB = _bcast_mid(tab_tile[:, tt, 128:192], 4)
            sB = _bcast_mid(tab_tile[:, tt, 192:256], 4)
            nc.vector.tensor_mul(t1[:], q1, cA)
            nc.vector.tensor_mul(t2[:], q2, sB)
            nc.vector.tensor_sub(qn[:, :, 0:64], t1[:], t2[:])
            nc.vector.tensor_mul(t3[:], q2, cB)
            nc.vector.tensor_mul(t4[:], q1, sA)
            nc.vector.tensor_add(qn[:, :, 64:128], t3[:], t4[:])

        def transpose4(qn, dst_ap):
            """PE-transpose 4 [128,128] heads of qn into dst_ap [128, 4, 128]."""
            tp = st_ps.tile([128, 512], BF16, tag="misc")
            for hh in range(4):
                nc.tensor.transpose(tp[:, hh * 128:(hh + 1) * 128], qn[:, hh, :], ident[:])
            nc.scalar.copy(out=dst_ap, in_=tp[:].rearrange("p (h t) -> p h t", h=4))

        # ---------------- P1: K/V projections ----------------
        # K transposes are deferred one tile behind the matmul stream so the
        # PE never waits for the DVE norm/rope tail.
        pend_k = None
        for c in range(4):
            wts = wts_first if c == 0 else load_w_tiles(wkvT, c * 512)
            for tt in range(8):
                if c == 0 and tt == 0:
                    hs_cb = hs_first
                else:
                    hs_cb = hp.tile([128, NDT, 128], BF16, tag="hs")
                    nc.sync.dma_start(out=hs_cb[:], in_=hs_kv[tt])
                ps = pp_ps.tile([128, 512], F32, tag="pp")
                for a in range(NDT):
                    nc.tensor.matmul(
                        ps[:], hs_cb[:, a, :], wts[a][:],
                        start=(a == 0), stop=(a == NDT - 1),
                    )
                if c < 2:  # K chunk: 4 kv heads c*4..c*4+3
                    kn = qnp.tile([128, 4, 128], BF16, tag="qqn")
                    norm_rope(ps, tabk, tt, kn)
                    if pend_k is not None:
                        transpose4(*pend_k)
                    pend_k = (kn, ktT[:, c * 4:(c + 1) * 4, tt * 128:(tt + 1) * 128])
                else:      # V chunk: plain bf16 copy
                    if pend_k is not None:
                        transpose4(*pend_k)
                        pend_k = None
                    nc.scalar.copy(
                        out=v_all[:, tt, (c - 2) * 4:(c - 1) * 4, :],
                        in_=ps[:].rearrange("p (h d) -> p h d", h=4),
                    )

        # ---------------- P2: Q projection + attention, per 4-head group ----
        # Software-pipelined: chunk c+1's projection matmuls are emitted
        # before chunk c's transposes+attention so the PE never waits for the
        # DVE norm/rope tail of the current chunk.
        def emit_q_proj(c):
            wts = load_w_tiles(wqT, c * 512)
            qns = []
            for qt in range(4):
                hs_cb = hp.tile([128, NDT, 128], BF16, tag="hs")
                nc.sync.dma_start(out=hs_cb[:], in_=hs_q[qt])
                ps = pp_ps.tile([128, 512], F32, tag="pp")
                for a in range(NDT):
                    nc.tensor.matmul(
                        ps[:], hs_cb[:, a, :], wts[a][:],
                        start=(a == 0), stop=(a == NDT - 1),
                    )
                qn = qnp.tile([128, 4, 128], BF16, tag="qqn")
                norm_rope(ps, tabq, qt, qn)
                qns.append(qn)
            return qns

        def emit_attention(c, qns):
            qTc = qtp.tile([128, 4, SQ], BF16, tag="qTc")  # [d, hh, q]
            for qt in range(4):
                transpose4(qns[qt], qTc[:, :, qt * 128:(qt + 1) * 128])
            for hh in range(4):
                h = c * 4 + hh
                hv = h // 4  # kv head (GQA group of 4)
                p_sb = attn_sb.tile([128, 8, 512], BF16, tag="p_sb")
                av = av_ps.tile([128, 512], F32, tag="av")
                for j in range(4):
                    st = st_ps.tile([128, 1024], F32, tag="misc")
                    for k in range(2):
                        kt = 2 * j + k
                        nc.tensor.matmul(
                            st[:, k * 512:(k + 1) * 512],
                            ktT[:, hv, kt * 128:(kt + 1) * 128], qTc[:, hh, :],
                            start=True, stop=True,
                        )
                    nc.scalar.activation(out=p_sb[:, 2 * j:2 * j + 2, :], in_=st[:],
                                         func=AF.Exp, scale=SCALE)
                # softmax denominator: sum p over the 8 key blocks (DVE), then
                # across partitions (GpSimd all-reduce), then 1/Z (DVE approx)
                acc = attn_sb.tile([128, 512], BF16, tag="acc")
                s01 = attn_sb.tile([128, 512], BF16, tag="s01")
                s23 = attn_sb.tile([128, 512], BF16, tag="s23")
                nc.vector.tensor_add(s01[:], p_sb[:, 0, :], p_sb[:, 1, :])
                nc.vector.tensor_add(s23[:], p_sb[:, 2, :], p_sb[:, 3, :])
                nc.vector.tensor_add(s01[:], s01[:], s23[:])
                nc.vector.tensor_add(acc[:], p_sb[:, 4, :], p_sb[:, 5, :])
                nc.vector.tensor_add(s23[:], p_sb[:, 6, :], p_sb[:, 7, :])
                nc.vector.tensor_add(acc[:], acc[:], s23[:])
                nc.vector.tensor_add(acc[:], acc[:], s01[:])
                zbc = attn_sb.tile([128, 512], F32, tag="zbc")
                nc.gpsimd.partition_all_reduce(out_ap=zbc[:], in_ap=acc[:],
                                               channels=128,
                                               reduce_op=bass_isa.ReduceOp.add)
                rz = attn_sb.tile([128, 512], F32, tag="rz")
                nc.vector.reciprocal_approx_fast(out=rz[:], in_=zbc[:])
                for kt in range(8):
                    nc.tensor.matmul(
                        av[:], v_all[:, kt, hv, :], p_sb[:, kt, :],
                        start=(kt == 0), stop=(kt == 7),
                    )
                nc.vector.tensor_mul(aT[:, h, :], av[:], rz[:])

        prev = None
        for c in range(8):
            qns = emit_q_proj(c)
            if prev is not None:
                emit_attention(prev[0], prev[1])
            prev = (c, qns)
        emit_attention(prev[0], prev[1])

        # ---------------- P3: o_proj ----------------
        for c in range(8):
            wts = load_w_tiles(woT, c * 512)
            for qt in range(4):
                ps = pp_ps.tile([128, 512], F32, tag="pp")
                for a in range(NDT):
                    nc.tensor.matmul(
                        ps[:], aT[:, a, qt * 128:(qt + 1) * 128], wts[a][:],
                        start=(a == 0), stop=(a == NDT - 1),
                    )
                yt = ysb.tile([128, 512], F32, tag="yt")
                nc.scalar.copy(out=yt[:], in_=ps[:])
                nc.sync.dma_start(
                    out=y[qt * 128:(qt + 1) * 128, c * 512:(c + 1) * 512], in_=yt[:]
                )

    nc.finalize()
    return nc


def _prep_inputs(inputs):
    pos = np.asarray(inputs["positions"]).astype(np.int32)
    hs = np.asarray(inputs["hidden_states"], dtype=np.float32)
    wq = np.asarray(inputs["wq"], dtype=np.float32)
    wk = np.asarray(inputs["wk"], dtype=np.float32)
    wv = np.asarray(inputs["wv"], dtype=np.float32)
    wo = np.asarray(inputs["wo"], dtype=np.float32)
    qw = np.asarray(inputs["q_norm_w"], dtype=np.float32)
    kw = np.asarray(inputs["k_norm_w"], dtype=np.float32)

    half = HD // 2
    inv_freq = (
        1.0 / (ROPE_BASE ** (np.arange(0, half, dtype=np.float32) * 2.0 / HD))
    ).astype(np.float32)
    ang = pos.astype(np.float32)[:, None] * inv_freq[None, :]  # [S, 64]
    cos = np.cos(ang).astype(np.float32)
    sin = np.sin(ang).astype(np.float32)

    def tab(w):
        w1, w2 = w[:half][None, :], w[half:][None, :]
        return np.ascontiguousarray(
            np.concatenate([cos * w1, sin * w1, cos * w2, sin * w2], axis=1)
        ).astype(np.float32)  # [S, 256] = [cA|sA|cB|sB]

    tq = tab(qw)
    tk = tab(kw)

    wkvT = np.ascontiguousarray(np.concatenate([wk, wv], axis=0).T).astype(_BF)
    wqT = np.ascontiguousarray(wq.T).astype(_BF)
    woT = np.ascontiguousarray(wo.T).astype(_BF)

    in_maps = []
    for core in range(N_CORES):
        b, qh = core // 2, core % 2
        hsb = np.ascontiguousarray(hs[b].T).astype(_BF)  # [4096, 1024]
        # [a*128+p, tt*128+t] -> [tt, p, a, t]
        hkv = np.ascontiguousarray(
            hsb.reshape(NDT, 128, 8, 128).transpose(2, 1, 0, 3)
        )
        hq = np.ascontiguousarray(
            hsb[:, qh * SQ:(qh + 1) * SQ].reshape(NDT, 128, 4, 128).transpose(2, 1, 0, 3)
        )
        in_maps.append(
            dict(
                hs_kv=hkv,
                hs_q=hq,
                wkvT=wkvT,
                wqT=wqT,
                woT=woT,
                ropeq=np.ascontiguousarray(tq[qh * SQ:(qh + 1) * SQ]),
                ropek=tk,
            )
        )
    return in_maps


_NC_CACHE = {}


def _get_nc():
    if "nc" not in _NC_CACHE:
        _NC_CACHE["nc"] = build_bass()
    return _NC_CACHE["nc"]


def _run(inputs, **spmd_kwargs):
    nc = _get_nc()
    in_maps = _prep_inputs(inputs)
    res = run_bass_kernel_spmd(nc, in_maps, list(range(N_CORES)), **spmd_kwargs)
    out = np.empty((B, S, HIDDEN), dtype=np.float32)
    for core in range(N_CORES):
        b, qh = core // 2, core % 2
        out[b, qh * SQ:(qh + 1) * SQ, :] = res.results[core]["y"]
    return out, res


def kernel(**inputs) -> np.ndarray:
    out, _ = _run(inputs)
    return out


if __name__ == "__main__":
    nc = build_bass()
    print("built OK:", len(nc.m.functions[0].blocks), "blocks")
